# revision 34
# baseline (speedup 1.0000x reference)
"""CTC loss (keras ctc_batch_cost semantics, full-length labels) on 8 TRN2 cores.

Pure data parallel: 16 examples/core. v2 design:

Gather (no PE transposes): y_pred is cast to bf16 on host; DMA loads each
(example, class-chunk) tile C-MAJOR as [128 classes, 512 t] (contiguous
256B segments along the class dim). One-hot matrices OH[c,s] (fwd) and
OH_rev[c,s] = OH[c,96-s] (bwd, s-reversed) are built on DVE from label
broadcasts; two accumulating bf16 matmuls per (ex, chunk) produce
ptF[s, t=0..255] and ptR[rev-s, t=256..511] in PSUM. ACT copies apply
P' = K*(p+eps) into the interleaved PPg buffers (bwd written with
negative t-stride so column u holds t=511-u); DVE applies the 0/1
skip masks for the z/w halves.

Scan: storing the backward state s-REVERSED makes both directions use
the SAME banded stationaries (A0T: diag+superdiag, Sh2T: s-2 skip):
  fwd:  comb[i] = a[i] + a[i-1] + z[i-2],  a' = comb*P'_t,   z' = a'*g
  bwd:  same recurrence on reversed q/w with reversed masks.
2 chains of 8 examples, each [97, 32] = [a(8) z(8) q(8) w(8)] bf16;
per step per chain: 2 accumulating matmuls (16 moving cols) + ONE DVE
multiply az' = rep2(PSUM)*PPg[:, u-block]. Renorm every 16 steps to
TGT=1e30 (per-example 1/c via PE broadcast, ln(c) deferred: c*CSC values
are stored and Ln'd once at the end).

Meet: comb_256 = A0T@a_255 + Sh2T@z_255; q_256 un-reversed via a
reversal matmul; two-path scaled dot + Ln with range clamps (HW Ln is
garbage outside ~(2^-66, 2^63)).
"""

import sys

for p in ("/opt/trn_rl_repo", "/root/.axon_site/_ro/trn_rl_repo"):
    if p not in sys.path:
        sys.path.insert(0, p)

import numpy as np
import ml_dtypes

import concourse.bass as bass
import concourse.bacc as bacc
import concourse.tile as tile
from concourse import mybir
from concourse.alu_op_type import AluOpType
from concourse.bass_utils import run_bass_kernel_spmd

F32 = mybir.dt.float32
BF16 = mybir.dt.bfloat16
FP8 = mybir.dt.float8e5
AF = mybir.ActivationFunctionType

N_CORES = 8
B, T, C, L = 128, 512, 1024, 48
S = 2 * L + 1  # 97
BC = B // N_CORES  # 16
EG = 8  # examples per chain
NCH = 2  # chains
TM = T // 2  # 256
BLANK = C - 1
KS = 512.0
EPS = 1e-7
TGT = 1e30
CSC = 2.0 ** -100
SCL1SQ = 2.0 ** -100
SCL2 = 2.0 ** 15
DSC1 = 2.0 ** 55
DSC2 = 2.0 ** 15
SWITCH = 1e-25
RN = 32
N_RENORM = 2 * (TM // RN - 1)  # 7 fwd + 7 bwd per example
NSL = 8  # ln-slot count (last slot stays 1.0)
_BIAS_COMMON = (
    T * np.log(KS) + (N_RENORM + 2) * np.log(TGT) + N_RENORM * np.log(CSC)
)
BIAS1 = float(_BIAS_COMMON + np.log(SCL1SQ) + np.log(DSC1))
BIAS2 = float(_BIAS_COMMON + 2 * np.log(SCL2) + np.log(DSC2))

_built = None


def _np_consts():
    # stationary St[k, i] = coefficient of x[k] in comb[i]
    a0t = np.zeros((S, S), np.float32)
    for k in range(S):
        a0t[k, k] = 1.0
        if k + 1 < S:
            a0t[k, k + 1] = 1.0
    sh2t = np.zeros((S, S), np.float32)
    for k in range(S - 2):
        sh2t[k, k + 2] = 1.0
    revt = np.zeros((S, S), np.float32)  # out[i] = in[96-i]
    for k in range(S):
        revt[k, S - 1 - k] = 1.0
    iota2 = np.zeros((128, 8 * S), np.float32)
    for c in range(8):
        iota2[:, c * S : (c + 1) * S] = (
            np.arange(128, dtype=np.float32)[:, None] + 128.0 * c
        )
    e01 = np.zeros((S, 4 * EG), np.float32)
    e01[0:2, :] = TGT
    ones_col = np.ones((S, 1), np.float32)
    ones_f = np.ones((S, 1), np.float32)
    tcol = np.full((1, S), TGT, np.float32)
    return a0t, sh2t, revt, iota2, e01, ones_col, ones_f, tcol


def _build():
    global _built
    if _built is not None:
        return _built

    a0t_np, sh2t_np, revt_np, iota2_np, e01_np, ones_np, onesf_np, tcol_np = (
        _np_consts()
    )

    nc = bacc.Bacc("TRN2")
    # host-pretransposed: [example, class, t] so each DMA partition row is
    # a contiguous 1KB run
    y = nc.dram_tensor("y_pred", [BC, C, T], FP8, kind="ExternalInput")
    extc = nc.dram_tensor("ext_c", [BC, S], F32, kind="ExternalInput")
    gf = nc.dram_tensor("gf", [S, BC], F32, kind="ExternalInput")
    grv = nc.dram_tensor("grv", [S, BC], F32, kind="ExternalInput")
    loss = nc.dram_tensor("loss", [BC, 1], F32, kind="ExternalOutput")

    cdefs = {
        "a0t": (a0t_np, BF16), "sh2t": (sh2t_np, BF16), "revt": (revt_np, BF16),
        "iota2": (iota2_np, F32), "e01": (e01_np, F32),
        "ones_col": (ones_np, BF16), "ones_f": (onesf_np, F32),
        "tcol": (tcol_np, F32),
    }
    handles = {
        k: nc.inline_tensor(v.astype(ml_dtypes.bfloat16) if dt == BF16 else v,
                            name=f"{k}_c")
        for k, (v, dt) in cdefs.items()
    }

    with tile.TileContext(nc) as tc:
        with (
            tc.tile_pool(name="consts", bufs=1) as consts,
            tc.tile_pool(name="persist", bufs=1) as persist,
            tc.tile_pool(name="ybufs", bufs=2) as ybufs,
            tc.tile_pool(name="ohp", bufs=2) as ohp,
            tc.tile_pool(name="azp", bufs=3) as azp,
            tc.tile_pool(name="small", bufs=4) as small,
            tc.tile_pool(name="pt_ps", bufs=1, space="PSUM") as pt_ps,
            tc.tile_pool(name="sc_ps", bufs=3, space="PSUM") as sc_ps,
            tc.tile_pool(name="rn_ps", bufs=1, space="PSUM") as rn_ps,
        ):
            cs = {}
            for k, (v, dt) in cdefs.items():
                cs[k] = consts.tile(list(v.shape), dt, tag=k, name=f"c_{k}")
                nc.sync.dma_start(out=cs[k], in_=handles[k].ap())
            masks = {}
            for k, h in (("gf", gf), ("grv", grv)):
                masks[k] = consts.tile([S, BC], F32, tag=k, name=f"m_{k}")
                nc.sync.dma_start(out=masks[k], in_=h.ap())

            # PPg per chain: 4*EG contiguous regions of [97, TM] bf16,
            # region (h, j) at offset (h*EG+j)*TM; h = 0:p_f 1:p_f*g
            # 2:p_b 3:p_b*sm
            ppg = [
                persist.tile([S, TM * 4 * EG], BF16, tag=f"ppg{x}", name=f"ppg_{x}")
                for x in range(NCH)
            ]
            # deferred-ln storage: NSL slots x [F(8)|B(8)]; last slot stays 1.0
            cst = [
                persist.tile([1, NSL * 2 * EG], F32, tag=f"cst{x}", name=f"cst_{x}")
                for x in range(NCH)
            ]
            for x in range(NCH):
                nc.vector.memset(cst[x], 1.0)

            y_ap = y.ap()
            extc_ap = extc.ap()

            # ---------------- gather ----------------
            for b in range(BC):
                x, j = divmod(b, EG)

                ext_rep = small.tile([128, S], F32, tag="ext_rep")
                src = bass.AP(
                    tensor=extc_ap.tensor,
                    offset=extc_ap.offset + b * S,
                    ap=[[0, 128], [1, S]],
                )
                nc.gpsimd.dma_start(out=ext_rep, in_=src)

                ytiles = []
                for q in range(8):
                    yt = ybufs.tile([128, T], FP8, tag=f"y{q}")
                    ysrc = bass.AP(
                        tensor=y_ap.tensor,
                        offset=y_ap.offset + b * C * T + q * 128 * T,
                        ap=[[T, 128], [1, T]],
                    )
                    nc.sync.dma_start(out=yt, in_=ysrc)
                    ytiles.append(yt)

                # one-hot (fwd only): one batched is_equal over 8 chunks
                oh = ohp.tile([128, 8 * S], FP8, tag="oh")
                ext_b = bass.AP(
                    tensor=ext_rep.tensor,
                    offset=ext_rep.offset,
                    ap=[ext_rep.ap[0], [0, 8], [1, S]],
                )
                nc.vector.tensor_tensor(
                    out=oh.rearrange("p (c s) -> p c s", c=8),
                    in0=ext_b, in1=cs["iota2"].rearrange("p (c s) -> p c s", c=8),
                    op=AluOpType.is_equal,
                )

                # full-width gather: pt[s, 0:512] in one accumulation
                pt = pt_ps.tile([S, T], F32, tag="ptF")
                for q in range(8):
                    nc.tensor.matmul(
                        pt, oh[:, q * S : (q + 1) * S], ytiles[q],
                        start=(q == 0), stop=(q == 7),
                    )

                # PPg writes: P' = KS*(p+eps), f32->bf16, contiguous regions
                pb = ppg[x]
                dstF = bass.AP(
                    tensor=pb.tensor, offset=pb.offset + j * TM,
                    ap=[pb.ap[0], [1, TM]],
                )
                nc.scalar.activation(
                    out=dstF, in_=pt[:, 0:TM], func=AF.Copy, scale=1.0,
                    bias=KS * EPS,
                )
                # bwd half: K-scale into bf16 staging, reverse s via matmul
                sbv = small.tile([S, TM], BF16, tag="sbv")
                nc.scalar.activation(
                    out=sbv, in_=pt[:, TM:T], func=AF.Copy, scale=1.0,
                    bias=KS * EPS,
                )
                psR = rn_ps.tile([S, TM], F32, tag="rn", name=f"psR_{b}")
                nc.tensor.matmul(psR, cs["revt"], sbv, start=True, stop=True)
                # psum col jj holds t = TM + jj -> u = 511 - t = 255 - jj
                dstR = bass.AP(
                    tensor=pb.tensor,
                    offset=pb.offset + (2 * EG + j) * TM + (TM - 1),
                    ap=[pb.ap[0], [-1, TM]],
                )
                nc.scalar.activation(
                    out=dstR, in_=psR, func=AF.Copy, scale=1.0,
                )

            # masked halves (batched over each chain's 8 examples)
            for x in range(NCH):
                pb = ppg[x]
                for half, mk in ((0, "gf"), (2, "grv")):
                    dst = bass.AP(
                        tensor=pb.tensor,
                        offset=pb.offset + (half + 1) * EG * TM,
                        ap=[pb.ap[0], [TM, EG], [1, TM]],
                    )
                    src_p = bass.AP(
                        tensor=pb.tensor, offset=pb.offset + half * EG * TM,
                        ap=[pb.ap[0], [TM, EG], [1, TM]],
                    )
                    mt = masks[mk]
                    m_b = bass.AP(
                        tensor=mt.tensor, offset=mt.offset + x * EG,
                        ap=[mt.ap[0], [1, EG], [0, TM]],
                    )
                    nc.vector.tensor_tensor(
                        out=dst, in0=src_p, in1=m_b, op=AluOpType.mult,
                    )

            # ---------------- scan ----------------
            def ppg_block(x, u):
                pb = ppg[x]
                return bass.AP(
                    tensor=pb.tensor, offset=pb.offset + u,
                    ap=[pb.ap[0], [2 * EG * TM, 2], [EG * TM, 2], [TM, EG]],
                )

            az = []
            for x in range(NCH):
                a0 = azp.tile([S, 4 * EG], BF16, tag=f"az{x}", name=f"az0_{x}")
                nc.vector.tensor_tensor(
                    out=a0.rearrange("p (a b c) -> p a b c", a=2, b=2),
                    in0=ppg_block(x, 0),
                    in1=cs["e01"].rearrange("p (a b c) -> p a b c", a=2, b=2),
                    op=AluOpType.mult,
                )
                az.append(a0)

            def az_mm_view(t, off):
                return bass.AP(
                    tensor=t.tensor, offset=t.offset + off,
                    ap=[t.ap[0], [2 * EG, 2], [1, EG]],
                )

            nslot = [0, 0]
            for u in range(1, TM):
                renorm = (u % RN) == 0
                ppg_in = [None, None]
                if renorm:
                    for x in range(NCH):
                        rn = rn_ps.tile([S, 4 * EG], F32, tag="rn",
                                        name=f"rn_{x}")
                        c_ps = rn[0:1, 2 * EG : 4 * EG]
                        nc.tensor.matmul(
                            c_ps, cs["ones_col"], az_mm_view(az[x], 0),
                            start=True, stop=True,
                        )
                        # store c*CSC for the deferred Ln
                        sl = nslot[x]
                        nslot[x] += 1
                        nc.vector.tensor_scalar(
                            out=cst[x][:, sl * 16 : sl * 16 + 16], in0=c_ps,
                            scalar1=CSC, scalar2=None, op0=AluOpType.mult,
                        )
                        r_sb = small.tile([1, 2 * EG], F32, tag=f"r{x}", name=f"r_{x}")
                        nc.vector.reciprocal(r_sb, c_ps)
                        rr = rn[:, 0 : 2 * EG]
                        nc.tensor.matmul(rr, cs["tcol"], r_sb, start=True, stop=True)
                        ppgs = small.tile([S, 4 * EG], BF16, tag=f"pps{x}",
                                          name=f"pps_{x}")
                        rr_rep = bass.AP(
                            tensor=rr.tensor, offset=rr.offset,
                            ap=[rr.ap[0], [EG, 2], [0, 2], [1, EG]],
                        )
                        nc.vector.tensor_tensor(
                            out=ppgs.rearrange("p (a b c) -> p a b c", a=2, b=2),
                            in0=ppg_block(x, u),
                            in1=rr_rep,
                            op=AluOpType.mult,
                        )
                        ppg_in[x] = ppgs.rearrange("p (a b c) -> p a b c",
                                                   a=2, b=2)
                else:
                    for x in range(NCH):
                        ppg_in[x] = ppg_block(x, u)

                for x in range(NCH):
                    p_ = sc_ps.tile([S, 2 * EG], F32, tag=f"ps{x}", name=f"ps_{x}")
                    nc.tensor.matmul(
                        p_, cs["a0t"], az_mm_view(az[x], 0),
                        start=True, stop=False,
                    )
                    nc.tensor.matmul(
                        p_, cs["sh2t"], az_mm_view(az[x], EG),
                        start=False, stop=True,
                    )
                    nw = azp.tile([S, 4 * EG], BF16, tag=f"az{x}", name=f"aznw_{x}")
                    rep = bass.AP(
                        tensor=p_.tensor, offset=p_.offset,
                        ap=[p_.ap[0], [EG, 2], [0, 2], [1, EG]],
                    )
                    nc.vector.tensor_tensor(
                        out=nw.rearrange("p (a b c) -> p a b c", a=2, b=2),
                        in0=rep,
                        in1=ppg_in[x],
                        op=AluOpType.mult,
                    )
                    az[x] = nw

            # ---------------- meet + finalize ----------------
            lsb = small.tile([1, BC], F32, tag="lsb")
            for x in range(NCH):
                comb = sc_ps.tile([S, EG], F32, tag=f"ps{x}", name=f"comb_{x}")
                nc.tensor.matmul(comb, cs["a0t"], az[x][:, 0:EG],
                                 start=True, stop=False)
                nc.tensor.matmul(comb, cs["sh2t"], az[x][:, EG : 2 * EG],
                                 start=False, stop=True)
                qn_ps = rn_ps.tile([S, EG], F32, tag="rn", name=f"qn_{x}")
                nc.tensor.matmul(qn_ps, cs["revt"], az[x][:, 2 * EG : 3 * EG],
                                 start=True, stop=True)
                qn = small.tile([S, EG], F32, tag="qn")
                nc.scalar.copy(out=qn, in_=qn_ps)

                # path 1 (shallow): product scaled by 2^-100 (fold onto q)
                q1 = small.tile([S, EG], F32, tag="q1")
                nc.vector.tensor_scalar(out=q1, in0=qn, scalar1=SCL1SQ,
                                        scalar2=None, op0=AluOpType.mult)
                m1 = small.tile([S, EG], F32, tag="m1")
                nc.vector.tensor_tensor(out=m1, in0=comb, in1=q1, op=AluOpType.mult)
                dot1 = sc_ps.tile([1, EG], F32, tag=f"ps{x}", name=f"dot1_{x}")
                nc.tensor.matmul(dot1, cs["ones_f"], m1, start=True, stop=True)
                # path 2 (deep): each factor scaled by 2^15 and clamped
                q2 = small.tile([S, EG], F32, tag="q2")
                nc.vector.tensor_scalar(out=q2, in0=qn, scalar1=SCL2,
                                        scalar2=None, op0=AluOpType.mult)
                c2 = small.tile([S, EG], F32, tag="c2")
                nc.scalar.activation(out=c2, in_=comb, func=AF.Copy, scale=SCL2)
                m2 = small.tile([S, EG], F32, tag="m2")
                nc.vector.tensor_tensor(out=m2, in0=c2, in1=q2, op=AluOpType.mult)
                m2c = small.tile([S, EG], F32, tag="m2c")
                nc.vector.tensor_scalar(out=m2c, in0=m2, scalar1=1e37,
                                        scalar2=None, op0=AluOpType.min)
                dot2 = rn_ps.tile([1, EG], F32, tag="rn", name=f"dot2_{x}")
                nc.tensor.matmul(dot2, cs["ones_f"], m2c, start=True, stop=True)

                # deferred ln of renorm sums: NSL slots (unused slots = ln 1 = 0)
                lnv = small.tile([1, NSL * 2 * EG], F32, tag="lnv")
                nc.scalar.activation(out=lnv, in_=cst[x], func=AF.Ln, scale=1.0)
                s1 = small.tile([1, 4 * 2 * EG], F32, tag="s1")
                nc.vector.tensor_tensor(out=s1, in0=lnv[:, 0 : 4 * 2 * EG],
                                        in1=lnv[:, 4 * 2 * EG : 8 * 2 * EG],
                                        op=AluOpType.add)
                s2 = small.tile([1, 2 * 2 * EG], F32, tag="s2")
                nc.vector.tensor_tensor(out=s2, in0=s1[:, 0 : 2 * 2 * EG],
                                        in1=s1[:, 2 * 2 * EG : 4 * 2 * EG],
                                        op=AluOpType.add)
                s4 = small.tile([1, 2 * EG], F32, tag="s4")
                nc.vector.tensor_tensor(out=s4, in0=s2[:, 0 : 2 * EG],
                                        in1=s2[:, 2 * EG : 4 * EG],
                                        op=AluOpType.add)
                lnT = small.tile([1, EG], F32, tag="lnT")
                nc.vector.tensor_tensor(out=lnT, in0=s4[:, 0:EG],
                                        in1=s4[:, EG : 2 * EG], op=AluOpType.add)

                # clamp both dots into HW-Ln-safe range (clamped branch is
                # never the selected one)
                d1c = small.tile([1, EG], F32, tag="d1c")
                nc.vector.tensor_scalar(out=d1c, in0=dot1, scalar1=1e-30,
                                        scalar2=2e2, op0=AluOpType.max,
                                        op1=AluOpType.min)
                d2c = small.tile([1, EG], F32, tag="d2c")
                nc.vector.tensor_scalar(out=d2c, in0=dot2, scalar1=1e-30,
                                        scalar2=1e14, op0=AluOpType.max,
                                        op1=AluOpType.min)
                lnd1 = small.tile([1, EG], F32, tag="lnd1")
                nc.scalar.activation(out=lnd1, in_=d1c, func=AF.Ln, scale=DSC1)
                lnd2 = small.tile([1, EG], F32, tag="lnd2")
                nc.scalar.activation(out=lnd2, in_=d2c, func=AF.Ln, scale=DSC2)
                cond = small.tile([1, EG], F32, tag="cond")
                nc.vector.tensor_scalar(out=cond, in0=d1c, scalar1=SWITCH,
                                        scalar2=None, op0=AluOpType.is_gt)
                u1 = small.tile([1, EG], F32, tag="u1")
                nc.vector.tensor_tensor(out=u1, in0=lnd1, in1=lnT, op=AluOpType.add)
                u2 = small.tile([1, EG], F32, tag="u2")
                nc.vector.tensor_tensor(out=u2, in0=lnd2, in1=lnT, op=AluOpType.add)
                v1 = small.tile([1, EG], F32, tag="v1")
                nc.scalar.activation(out=v1, in_=u1, func=AF.Copy, scale=-1.0,
                                     bias=BIAS1)
                v2 = small.tile([1, EG], F32, tag="v2")
                nc.scalar.activation(out=v2, in_=u2, func=AF.Copy, scale=-1.0,
                                     bias=BIAS2)
                dv = small.tile([1, EG], F32, tag="dv")
                nc.vector.tensor_tensor(out=dv, in0=v1, in1=v2,
                                        op=AluOpType.subtract)
                cd = small.tile([1, EG], F32, tag="cd")
                nc.vector.tensor_tensor(out=cd, in0=cond, in1=dv, op=AluOpType.mult)
                nc.vector.tensor_tensor(
                    out=lsb[:, x * EG : (x + 1) * EG],
                    in0=v2, in1=cd, op=AluOpType.add,
                )
            nc.sync.dma_start(out=loss.ap().rearrange("b o -> o b"), in_=lsb)

    nc.compile()
    _built = nc
    return nc


def _host_prep(y_pred: np.ndarray, y_true: np.ndarray):
    y_true = np.asarray(y_true).astype(np.int64)
    ext = np.full((B, S), BLANK, np.int64)
    ext[:, 1::2] = y_true
    ext_m2 = np.concatenate([np.full((B, 2), -1, np.int64), ext[:, :-2]], axis=1)
    skip = (ext != BLANK) & (ext != ext_m2)  # [B,S]
    g = np.zeros((B, S), np.float32)
    g[:, :-2] = skip[:, 2:].astype(np.float32)  # fwd z-mask: g[s] = skip[s+2]
    smr = skip[:, ::-1].astype(np.float32)  # bwd w-mask, s-reversed
    extc = ext.astype(np.float32)

    y16 = (np.asarray(y_pred) * KS).astype(ml_dtypes.float8_e5m2)

    in_maps = []
    for core in range(N_CORES):
        sl_ = slice(core * BC, (core + 1) * BC)
        in_maps.append(
            {
                "y_pred": np.ascontiguousarray(
                    np.transpose(y16[sl_], (0, 2, 1))),
                "ext_c": np.ascontiguousarray(extc[sl_]),
                "gf": np.ascontiguousarray(g[sl_].T),
                "grv": np.ascontiguousarray(smr[sl_].T),
            }
        )
    return in_maps


def kernel(y_pred: np.ndarray, y_true: np.ndarray) -> np.ndarray:
    nc = _build()
    in_maps = _host_prep(y_pred, y_true)
    res = run_bass_kernel_spmd(nc, in_maps, core_ids=list(range(N_CORES)))
    out = np.concatenate([r["loss"] for r in res.results], axis=0)
    return out.astype(np.float32)
